# revision 14
# baseline (speedup 1.0000x reference)
"""Cross bi-directional Mamba block (DirectionalAGLGF) on 8 Trainium2 cores.

Sharding: (batch 2) x (sequence-quarter 4). The SSM scan is sequence-parallel
with a 128-step decay warmup instead of cross-core state handoff (state decays
by >= exp(-23) over the warmup window, far below fp32 resolution).

Per-core layout: features on partitions, sequence on the free dimension.
  - LN folded into projection weights; stats via PE ones-matmuls; rsqrt via
    exp(-0.5*ln(v)) (single ACT table set: {exp, ln, square, copy, identity}).
  - causal conv folded into the input projection (4 shifted accumulating
    matmuls with conv-premultiplied weights).
  - silu(x) = x * exp(-ln(1+exp(-x))), softplus(x) = ln(1+exp(x)).
  - scan state tiles pack 32 d-channels x 4 n-channels per 128 partitions;
    dt/dtu expanded across n by 0/1 matmuls (fp32r), B/C expanded across d
    by replicating DMA reads.
  - recurrence via the DVE tensor_tensor_scan instruction (reversed access
    patterns for the backward direction - no flips materialized).
  - y = sum_n C*h via block-ones matmul accumulated over n-quarters in PSUM.
"""
import sys
sys.path.insert(0, '/opt/trn_rl_repo')
sys.path.insert(0, '/root/.axon_site/_ro/trn_rl_repo')
import numpy as np

B, C, HW, L = 2, 128, 64, 4096
D, N, R, K = 256, 16, 8, 4
Lo, W = 1024, 128
SW = Lo + W            # scan window 1152
XW = Lo + 2 * W + 16   # x1n window 1296
CHUNKS = [(0, 512), (512, 512), (1024, SW - 1024)]
XCH = [(0, 512), (512, 512), (1024, XW - 1024)]
OCH = [(0, 512), (512, 512)]
DT_OFF = [8, 136]          # x1n idx of scan-window j=0, per dir
TAP_OFF = [[5, 6, 7, 8], [139, 138, 137, 136]]  # +k fwd, +3-k bwd
OWN_OFF = [W, 0]           # owned slice start within scan window

_STATE = {}


def _prep_params(p):
    """Host-side parameter folding (numpy, tiny)."""
    f32 = np.float32
    out = {}
    ln_q_w, ln_q_b = p['ln_q_w'], p['ln_q_b']
    ln_kv_w, ln_kv_b = p['ln_kv_w'], p['ln_kv_b']
    w_in_x, w_in_z = p['w_in_x'], p['w_in_z']
    conv_w = [p['conv_w'], p['conv_w_b']]
    conv_b = [p['conv_b'], p['conv_b_b']]
    xpw = [p['x_proj_w'], p['x_proj_w_b']]
    dtw = [p['dt_w'], p['dt_w_b']]
    dtb = [p['dt_b'], p['dt_b_b']]
    A_log = [p['A_log'], p['A_log_b']]
    Dp = [p['D'], p['D_b']]

    wx_ln = w_in_x * ln_q_w[None, :]          # (256,128)
    t_x = w_in_x @ ln_q_b                     # (256,)
    wG = np.zeros((2, K, 128, D), f32)        # lhsT (c, d) per dir,k
    bias_x = np.zeros((2, 2, 128, 1), f32)    # (dir, dchunk, 128, 1)
    for dr in range(2):
        for k in range(K):
            wG[dr, k] = (conv_w[dr][:, k:k + 1] * wx_ln).T
        bx = conv_b[dr] + t_x * conv_w[dr].sum(axis=1)
        bias_x[dr] = bx.reshape(2, 128, 1)
    out['wG'] = wG
    out['bias_x'] = bias_x
    out['neg_bias_x'] = -bias_x
    out['wZ'] = (w_in_z * ln_kv_w[None, :]).T.astype(f32).copy()   # (128,256)
    bz = (w_in_z @ ln_kv_b).astype(f32)
    out['bias_z'] = bz.reshape(2, 128, 1)
    out['neg_bias_z'] = -bz.reshape(2, 128, 1)
    out['xpwT'] = np.stack([w.T for w in xpw]).astype(f32)         # (2,256,40)
    out['dtwT'] = np.stack([w.T for w in dtw]).astype(f32)         # (2,8,256)
    out['dtb'] = np.stack(dtb).astype(f32).reshape(2, 2, 128, 1)
    A = [-np.exp(a).astype(f32) for a in A_log]                    # (256,16)
    acols = np.zeros((2, 128, 32), f32)
    pidx = np.arange(128)
    for dr in range(2):
        for t in range(32):
            g, nq = t // 4, t % 4
            acols[dr, :, t] = A[dr][32 * g + pidx % 32, 4 * nq + pidx // 32]
    out['A_cols'] = acols
    eq = np.zeros((128, 512), f32)
    for gq in range(4):
        for pp in range(128):
            eq[32 * gq + pp % 32, 128 * gq + pp] = 1.0
    out['Eq'] = eq
    ones_red = np.zeros((128, 32), f32)
    ones_red[pidx, pidx % 32] = 1.0
    out['ones_red'] = ones_red
    out['outwT'] = p['out_w'].T.astype(f32).copy()                 # (256,128)
    out['out_b'] = p['out_b'].astype(f32).reshape(128, 1)
    out['D_cols'] = np.stack(Dp).astype(f32).reshape(2, 2, 128, 1)
    return out


def _owned_chunks(dr):
    """Map scan-window chunks to owned-range [0,Lo) column spans.
    Returns list of (chunk_idx, src_lo, src_len, dst_off)."""
    oo = OWN_OFF[dr]
    res = []
    for ci, (s, ln) in enumerate(CHUNKS):
        a = max(s, oo)
        bnd = min(s + ln, oo + Lo)
        if a < bnd:
            res.append((ci, a - s, bnd - a, a - oo))
    return res


def _build(nc):
    import concourse.mybir as mybir
    import concourse.tile as tile
    f32 = mybir.dt.float32
    f32r = mybir.dt.float32r
    Alu = mybir.AluOpType
    AF = mybir.ActivationFunctionType
    Exp, Ln, Sq, Ident = AF.Exp, AF.Ln, AF.Square, AF.Identity

    dp = nc.declare_dram_parameter
    d_x1 = dp("x1s", [128, XW], f32, isOutput=False)
    d_x2 = dp("x2s", [128, XW], f32, isOutput=False)
    d_mf = dp("mask_f", [128, 512], f32, isOutput=False)
    d_mb = dp("mask_b", [128, SW - 1024], f32, isOutput=False)
    d_wG = dp("wG", [2, K, 128, D], f32, isOutput=False)
    d_bx = dp("bias_x", [2, 2, 128, 1], f32, isOutput=False)
    d_nbx = dp("neg_bias_x", [2, 2, 128, 1], f32, isOutput=False)
    d_wZ = dp("wZ", [128, D], f32, isOutput=False)
    d_bz = dp("bias_z", [2, 128, 1], f32, isOutput=False)
    d_nbz = dp("neg_bias_z", [2, 128, 1], f32, isOutput=False)
    d_xpwT = dp("xpwT", [2, D, 40], f32, isOutput=False)
    d_dtwT = dp("dtwT", [2, R, D], f32, isOutput=False)
    d_dtb = dp("dtb", [2, 2, 128, 1], f32, isOutput=False)
    d_ac = dp("A_cols", [2, 128, 32], f32, isOutput=False)
    d_eq = dp("Eq", [128, 512], f32, isOutput=False)
    d_or = dp("ones_red", [128, 32], f32, isOutput=False)
    d_ow = dp("outwT", [D, 128], f32, isOutput=False)
    d_ob = dp("out_b", [128, 1], f32, isOutput=False)
    d_dc = dp("D_cols", [2, 2, 128, 1], f32, isOutput=False)
    d_out = dp("out", [128, Lo], f32, isOutput=True)

    with tile.TileContext(nc) as tc:
        with (tc.tile_pool(name="cp", bufs=1) as cp,
              tc.tile_pool(name="mp", bufs=1) as mp,
              tc.tile_pool(name="ps", bufs=1, space="PSUM") as ps):

            def t5(name):
                return mp.tile([128, 512], f32, name=name, tag="tmp5", bufs=3)

            # ---------------- weights / consts ----------------
            def load_r(name, shape, src_ap):
                """DMA f32 -> staging, DVE copy -> f32r tile."""
                stg = mp.tile([128, 512], f32, name=f"stg_{name}", tag="tmp5", bufs=3)
                nc.sync.dma_start(stg[:shape[0], :shape[1]], src_ap)
                t = cp.tile(list(shape), f32r, name=name)
                nc.vector.tensor_copy(t[:], stg[:shape[0], :shape[1]])
                return t

            wG_t = [[[load_r(f"wG{dr}{k}{dc}", (128, 128),
                             d_wG[dr, k, :, 128 * dc:128 * dc + 128])
                      for dc in range(2)] for k in range(K)] for dr in range(2)]
            wZ_t = [load_r(f"wZ{dc}", (128, 128), d_wZ[:, 128 * dc:128 * dc + 128])
                    for dc in range(2)]
            xpwT_t = [[load_r(f"xpw{dr}{dc}", (128, 40),
                              d_xpwT[dr, 128 * dc:128 * dc + 128, :])
                       for dc in range(2)] for dr in range(2)]
            dtwT_t = [[load_r(f"dtw{dr}{dc}", (R, 128),
                              d_dtwT[dr, :, 128 * dc:128 * dc + 128])
                       for dc in range(2)] for dr in range(2)]
            eq_t = load_r("eqt", (128, 512), d_eq[:, :])
            or_t = load_r("ort", (128, 32), d_or[:, :])
            ow_t = [load_r(f"ow{dc}", (128, 128), d_ow[128 * dc:128 * dc + 128, :])
                    for dc in range(2)]

            def load_f(name, shape, src_ap):
                t = cp.tile(list(shape), f32, name=name)
                nc.sync.dma_start(t[:], src_ap)
                return t

            bz_t = [load_f(f"bzt{dc}", (128, 1), d_bz[dc, :, :]) for dc in range(2)]
            nbz_t = [load_f(f"nbzt{dc}", (128, 1), d_nbz[dc, :, :]) for dc in range(2)]
            dtb_t = [[load_f(f"dtbt{dr}{dc}", (128, 1), d_dtb[dr, dc, :, :])
                      for dc in range(2)] for dr in range(2)]
            bx_t = [[load_f(f"bxt{dr}{dc}", (128, 1), d_bx[dr, dc, :, :])
                     for dc in range(2)] for dr in range(2)]
            nbx_t = [[load_f(f"nbxt{dr}{dc}", (128, 1), d_nbx[dr, dc, :, :])
                      for dc in range(2)] for dr in range(2)]
            ac_t = [load_f(f"act{dr}", (128, 32), d_ac[dr, :, :]) for dr in range(2)]
            dc_t = [[load_f(f"dct{dr}{dc}", (128, 1), d_dc[dr, dc, :, :])
                     for dc in range(2)] for dr in range(2)]
            ob_t = load_f("obt", (128, 1), d_ob[:, :])
            mf_t = load_f("mft", (128, 512), d_mf[:, :])
            mb_t = load_f("mbt", (128, SW - 1024), d_mb[:, :])
            ones1 = cp.tile([128, 1], f32, name="ones1")
            nc.vector.memset(ones1[:], 1.0)
            eps_t = cp.tile([128, 1], f32, name="eps_t")
            nc.vector.memset(eps_t[:], 1e-5)

            # ---------------- layernorm ----------------
            def rowc(name):
                return mp.tile([1, 512], f32, name=name, tag="rowc", bufs=5)

            def layernorm(d_in, out_name):
                raw = mp.tile([128, XW], f32, name=f"raw_{out_name}", tag="w1296", bufs=3)
                nc.sync.dma_start(raw[:], d_in[:, :])
                rb = mp.tile([128, XW], f32, name=f"rb_{out_name}", tag="w1296", bufs=3)
                murb = mp.tile([128, XW], f32, name=f"murb_{out_name}", tag="w1296", bufs=3)
                for (s, ln) in XCH:
                    sq = t5(f"sq_{out_name}{s}")
                    nc.scalar.activation(sq[:, :ln], raw[:, s:s + ln], Sq)
                    p1 = ps.tile([1, 512], f32, name=f"pst1_{out_name}{s}", tag="red", bufs=2)
                    p2 = ps.tile([1, 512], f32, name=f"pst2_{out_name}{s}", tag="red", bufs=2)
                    nc.tensor.matmul(p1[:, :ln], ones1[:], raw[:, s:s + ln],
                                     start=True, stop=True)
                    nc.tensor.matmul(p2[:, :ln], ones1[:], sq[:, :ln],
                                     start=True, stop=True)
                    mu = rowc(f"mu_{out_name}{s}")
                    msq = rowc(f"msq_{out_name}{s}")
                    nc.scalar.mul(mu[:, :ln], p1[:, :ln], 1.0 / 128)
                    nc.scalar.mul(msq[:, :ln], p2[:, :ln], 1.0 / 128)
                    mu2 = rowc(f"mu2_{out_name}{s}")
                    nc.scalar.activation(mu2[:, :ln], mu[:, :ln], Sq)
                    var = rowc(f"var_{out_name}{s}")
                    nc.vector.tensor_tensor(var[:, :ln], msq[:, :ln], mu2[:, :ln],
                                            Alu.subtract)
                    lnv = rowc(f"lnv_{out_name}{s}")
                    nc.scalar.activation(lnv[:, :ln], var[:, :ln], Ln, bias=eps_t[:1, :])
                    r = rowc(f"r_{out_name}{s}")
                    nc.scalar.activation(r[:, :ln], lnv[:, :ln], Exp, scale=-0.5)
                    mur = rowc(f"mur_{out_name}{s}")
                    nc.vector.tensor_tensor(mur[:, :ln], mu[:, :ln], r[:, :ln],
                                            Alu.mult)
                    nc.gpsimd.partition_broadcast(rb[:, s:s + ln], r[:, :ln])
                    nc.gpsimd.partition_broadcast(murb[:, s:s + ln], mur[:, :ln])
                xn = mp.tile([128, XW], f32r, name=out_name, tag="xn", bufs=2)
                for (s, ln) in XCH:
                    tmp = t5(f"tmpn_{out_name}{s}")
                    nc.vector.tensor_tensor(tmp[:, :ln], raw[:, s:s + ln],
                                            rb[:, s:s + ln], Alu.mult)
                    nc.vector.tensor_tensor(xn[:, s:s + ln], tmp[:, :ln],
                                            murb[:, s:s + ln], Alu.subtract)
                return xn

            x1n = layernorm(d_x1, "x1n")
            x2n = layernorm(d_x2, "x2n")

            # ---------------- z branch: zs = silu(z) ----------------
            zs = [mp.tile([128, Lo], f32, name=f"zs{dc}", tag="zs", bufs=2)
                  for dc in range(2)]
            for dc in range(2):
                for (s, ln) in OCH:
                    pz = ps.tile([128, 512], f32, name=f"pz{dc}{s}", tag="mm", bufs=2)
                    nc.tensor.matmul(pz[:, :ln], wZ_t[dc][:],
                                     x2n[:, 136 + s:136 + s + ln], start=True, stop=True)
                    e = t5(f"ze{dc}{s}")
                    nc.scalar.activation(e[:, :ln], pz[:, :ln], Exp, scale=-1.0,
                                         bias=nbz_t[dc][:])
                    sp = t5(f"zsp{dc}{s}")
                    nc.scalar.activation(sp[:, :ln], e[:, :ln], Ln, bias=1.0)
                    sg = t5(f"zsg{dc}{s}")
                    nc.scalar.activation(sg[:, :ln], sp[:, :ln], Exp, scale=-1.0)
                    nc.vector.scalar_tensor_tensor(
                        zs[dc][:, s:s + ln], pz[:, :ln], bz_t[dc][:],
                        sg[:, :ln], Alu.add, Alu.mult)

            # ---------------- per-direction pipeline ----------------
            ysum = [None, None]     # accumulated (y + u*D) over directions
            for dr in range(2):
                och = _owned_chunks(dr)
                # --- conv-folded input projection + silu -> xc chunks;
                #     u*D accumulated early into ud tiles ---
                xcc = [[None] * len(CHUNKS) for _ in range(2)]
                ud = [mp.tile([128, Lo], f32, name=f"ud{dr}{dc}", tag="ud", bufs=2)
                      for dc in range(2)]
                for dc in range(2):
                    for ci, (s, ln) in enumerate(CHUNKS):
                        px = ps.tile([128, 512], f32, name=f"px{dr}{dc}{s}", tag="mm", bufs=2)
                        for k in range(K):
                            t0 = TAP_OFF[dr][k] + s
                            nc.tensor.matmul(px[:, :ln], wG_t[dr][k][dc][:],
                                             x1n[:, t0:t0 + ln],
                                             start=(k == 0), stop=(k == K - 1))
                        e = t5(f"xe{dr}{dc}{s}")
                        nc.scalar.activation(e[:, :ln], px[:, :ln], Exp, scale=-1.0,
                                             bias=nbx_t[dr][dc][:])
                        sp = t5(f"xsp{dr}{dc}{s}")
                        nc.scalar.activation(sp[:, :ln], e[:, :ln], Ln, bias=1.0)
                        sg = t5(f"xsg{dr}{dc}{s}")
                        nc.scalar.activation(sg[:, :ln], sp[:, :ln], Exp, scale=-1.0)
                        xc = mp.tile([128, 512], f32r, name=f"xc{dr}{dc}{s}",
                                     tag="xcc", bufs=6)
                        nc.vector.scalar_tensor_tensor(
                            xc[:, :ln], px[:, :ln], bx_t[dr][dc][:],
                            sg[:, :ln], Alu.add, Alu.mult)
                        xcc[dc][ci] = xc
                    # u*D on owned sub-spans of each chunk
                    for (ci, slo, sln, doff) in och:
                        nc.vector.tensor_scalar_mul(
                            ud[dc][:, doff:doff + sln],
                            xcc[dc][ci][:, slo:slo + sln], dc_t[dr][dc][:])

                # --- x_proj -> dbl (dt_r 8 | B 16 | C 16) ---
                dbl = mp.tile([40, SW], f32r, name=f"dbl{dr}", tag="dbl", bufs=1)
                for ci, (s, ln) in enumerate(CHUNKS):
                    p40 = ps.tile([40, 512], f32, name=f"p40_{dr}{s}", tag="mm", bufs=2)
                    for dc in range(2):
                        nc.tensor.matmul(p40[:, :ln], xpwT_t[dr][dc][:],
                                         xcc[dc][ci][:, :ln],
                                         start=(dc == 0), stop=(dc == 1))
                    nc.scalar.copy(dbl[:, s:s + ln], p40[:, :ln])

                # --- B_exp / C_exp by replicating DMA (src-major) ---
                oo = OWN_OFF[dr]
                bexp, cexp = [], []
                for nq in range(4):
                    bx = mp.tile([128, SW], f32, name=f"bex{dr}{nq}", tag="bex", bufs=4)
                    src = dbl[8 + 4 * nq:12 + 4 * nq, :].bitcast(f32)
                    nc.scalar.dma_start(bx[:], src.unsqueeze(1).to_broadcast((4, 32, SW)))
                    bexp.append(bx)
                    cx = mp.tile([128, Lo], f32, name=f"cex{dr}{nq}", tag="cex", bufs=4)
                    csrc = dbl[24 + 4 * nq:28 + 4 * nq, oo:oo + Lo].bitcast(f32)
                    nc.scalar.dma_start(cx[:], csrc.unsqueeze(1).to_broadcast((4, 32, Lo)))
                    cexp.append(cx)

                # --- per d-chunk: dt/dtu chunks, then its 4 groups ---
                ydir = [mp.tile([128, Lo], f32, name=f"yd{dr}{dc}", tag="ydir", bufs=2)
                        for dc in range(2)]
                for dc in range(2):
                    dtt, dtu = [], []
                    for ci, (s, ln) in enumerate(CHUNKS):
                        pd = ps.tile([128, 512], f32, name=f"pd{dr}{dc}{s}", tag="mm", bufs=2)
                        nc.tensor.matmul(pd[:, :ln], dtwT_t[dr][dc][:],
                                         dbl[0:8, s:s + ln], start=True, stop=True)
                        e = t5(f"de{dr}{dc}{s}")
                        nc.scalar.activation(e[:, :ln], pd[:, :ln], Exp,
                                             bias=dtb_t[dr][dc][:])
                        dt_c = mp.tile([128, 512], f32r, name=f"dt{dr}{dc}{s}",
                                       tag="dtc", bufs=4)
                        masked = (dr == 0 and ci == 0) or (dr == 1 and ci == 2)
                        if masked:
                            spt = t5(f"dsp{dr}{dc}{s}")
                            nc.scalar.activation(spt[:, :ln], e[:, :ln], Ln, bias=1.0)
                            mt = mf_t if dr == 0 else mb_t
                            nc.vector.tensor_tensor(dt_c[:, :ln], spt[:, :ln],
                                                    mt[:, :ln], Alu.mult)
                        else:
                            nc.scalar.activation(dt_c[:, :ln], e[:, :ln], Ln, bias=1.0)
                        du_c = mp.tile([128, 512], f32r, name=f"du{dr}{dc}{s}",
                                       tag="duc", bufs=4)
                        nc.vector.tensor_tensor(du_c[:, :ln], dt_c[:, :ln],
                                                xcc[dc][ci][:, :ln], Alu.mult)
                        dtt.append(dt_c)
                        dtu.append(du_c)

                    for gq in range(4):
                        g = 4 * dc + gq
                        pe_dt = []
                        due_s = mp.tile([128, SW], f32, name=f"due{dr}{g}",
                                        tag="due", bufs=2)
                        for ci, (s, ln) in enumerate(CHUNKS):
                            pdt = ps.tile([128, 512], f32, name=f"pdt{dr}{g}{s}",
                                          tag="exp", bufs=4)
                            nc.tensor.matmul(pdt[:, :ln],
                                             eq_t[:, 128 * gq:128 * gq + 128],
                                             dtt[ci][:, :ln], start=True, stop=True)
                            pe_dt.append(pdt)
                            pdu = ps.tile([128, 512], f32, name=f"pdu{dr}{g}{s}",
                                          tag="exp", bufs=4)
                            nc.tensor.matmul(pdu[:, :ln],
                                             eq_t[:, 128 * gq:128 * gq + 128],
                                             dtu[ci][:, :ln], start=True, stop=True)
                            nc.scalar.copy(due_s[:, s:s + ln], pdu[:, :ln])
                        red = [ps.tile([32, 512], f32, name=f"red{dr}{g}{lc}",
                                       tag="red", bufs=2) for lc in range(2)]
                        for nq in range(4):
                            t = g * 4 + nq
                            dA = mp.tile([128, SW], f32, name=f"dA{dr}{t}",
                                         tag="dA", bufs=2)
                            for ci, (s, ln) in enumerate(CHUNKS):
                                nc.scalar.activation(dA[:, s:s + ln], pe_dt[ci][:, :ln],
                                                     Exp, scale=ac_t[dr][:, t:t + 1])
                            dB = mp.tile([128, SW], f32, name=f"dB{dr}{t}",
                                         tag="dB", bufs=2)
                            nc.vector.tensor_tensor(dB[:], due_s[:], bexp[nq][:],
                                                    Alu.mult)
                            # scan in-place: hs overwrites dB (per-element
                            # read-then-write, state kept in-engine)
                            if dr == 0:
                                nc.vector.tensor_tensor_scan(dB[:], dA[:], dB[:], 0.0,
                                                             Alu.mult, Alu.add)
                            else:
                                nc.vector.tensor_tensor_scan(dB[:, ::-1], dA[:, ::-1],
                                                             dB[:, ::-1], 0.0,
                                                             Alu.mult, Alu.add)
                            pr = mp.tile([128, Lo], f32r, name=f"pr{dr}{t}",
                                         tag="pr", bufs=2)
                            nc.vector.tensor_tensor(pr[:], dB[:, oo:oo + Lo],
                                                    cexp[nq][:], Alu.mult)
                            for lc in range(2):
                                nc.tensor.matmul(red[lc][:, :], or_t[:],
                                                 pr[:, 512 * lc:512 * lc + 512],
                                                 start=(nq == 0), stop=(nq == 3))
                        for lc in range(2):
                            nc.scalar.copy(
                                ydir[dc][32 * gq:32 * gq + 32, 512 * lc:512 * lc + 512],
                                red[lc][:, :])

                # --- accumulate y_dir + u*D over directions ---
                for dc in range(2):
                    if dr == 0:
                        a1 = mp.tile([128, Lo], f32, name=f"a1{dc}", tag="accp", bufs=2)
                        nc.vector.tensor_tensor(a1[:], ydir[dc][:], ud[dc][:], Alu.add)
                        ysum[dc] = a1
                    else:
                        nc.vector.tensor_tensor(ud[dc][:], ud[dc][:], ydir[dc][:],
                                                Alu.add)
                        nc.vector.tensor_tensor(ysum[dc][:], ysum[dc][:], ud[dc][:],
                                                Alu.add)

            # ---------------- gate + output projection ----------------
            outs = mp.tile([128, Lo], f32, name="outs")
            yg = [None, None]
            for dc in range(2):
                yg[dc] = mp.tile([128, Lo], f32r, name=f"yg{dc}", tag="yg", bufs=2)
                nc.vector.tensor_tensor(yg[dc][:], ysum[dc][:], zs[dc][:], Alu.mult)
            for (s, ln) in OCH:
                po = ps.tile([128, 512], f32, name=f"po{s}", tag="mm", bufs=2)
                for dc in range(2):
                    nc.tensor.matmul(po[:, :ln], ow_t[dc][:], yg[dc][:, s:s + ln],
                                     start=(dc == 0), stop=(dc == 1))
                nc.scalar.activation(outs[:, s:s + ln], po[:, :ln], Ident, bias=ob_t[:])
            nc.sync.dma_start(d_out[:, :], outs[:])
    return nc


def kernel(**inputs):
    import concourse.bacc as bacc
    from concourse.bass_utils import run_bass_kernel_spmd

    x1, x2 = inputs['x1'], inputs['x2']
    params = _prep_params(inputs)

    if 'nc' not in _STATE:
        nc = bacc.Bacc("TRN2", target_bir_lowering=False, debug=False)
        _build(nc)
        nc.compile()
        _STATE['nc'] = nc
    nc = _STATE['nc']

    x1f = np.ascontiguousarray(x1.reshape(B, 128, L)).astype(np.float32)
    x2f = np.ascontiguousarray(x2.reshape(B, 128, L)).astype(np.float32)

    in_maps = []
    for core in range(8):
        b, q = core // 4, core % 4
        lo = 1024 * q - (W + 8)
        sl = np.zeros((2, 128, XW), np.float32)
        a, bnd = max(0, lo), min(L, lo + XW)
        sl[0][:, a - lo:bnd - lo] = x1f[b][:, a:bnd]
        sl[1][:, a - lo:bnd - lo] = x2f[b][:, a:bnd]
        idx = lo + np.arange(XW)
        valid = ((idx >= 0) & (idx < L)).astype(np.float32)
        mask_f = np.broadcast_to(valid[8:520], (128, 512)).copy()
        mask_b = np.broadcast_to(valid[1160:1160 + SW - 1024], (128, SW - 1024)).copy()
        m = {"x1s": sl[0], "x2s": sl[1], "mask_f": mask_f, "mask_b": mask_b}
        m.update(params)
        in_maps.append(m)

    res = run_bass_kernel_spmd(nc, in_maps, list(range(8))).results
    out = np.zeros((B, 128, L), np.float32)
    for core in range(8):
        b, q = core // 4, core % 4
        out[b][:, 1024 * q:1024 * (q + 1)] = res[core]["out"]
    return out.reshape(B, 128, HW, HW), x2
